# revision 1
# baseline (speedup 1.0000x reference)
"""Multi-head causal attention (B=4, T=2048, C=1024, H=16, D=64) on 8 trn2 cores.

Sharding: tensor-parallel over heads within batch core-pairs.
  core c -> batch b = c//2, heads hoff..hoff+7 where hoff = (c%2)*8.
Each core:
  - projects Q^T/K^T (head-pairs packed to 128 partitions) and V (head-quads
    packed, stride-65 layout with a ones column folded in for free softmax sums)
  - causal attention per head in S^T = [j, i] orientation, exp without
    max-subtraction (scores are ~N(0, 0.25^2), safe), fp32r matmuls throughout
  - output projection to partial y^T [1024 c', 2048 t] (+ bo/2)
  - pairwise ReduceScatter (4 t-slabs) sums partner partials; core even keeps
    c' 0:512, odd keeps c' 512:1024.
Host reassembles the [B, T, C] output by transposing/concatenating slabs.
"""

import numpy as np

import concourse.bass as bass
import concourse.mybir as mybir
from concourse import bacc
from concourse.tile import TileContext
from concourse.bass_utils import run_bass_kernel_spmd

F32 = mybir.dt.float32
F32R = mybir.dt.float32r

B, T, C = 4, 2048, 1024
H, D = 16, 64
HC = 8           # heads per core
NPAIR = HC // 2  # head pairs (QK packing)
CCn = C // 128   # 8 contraction chunks
TTn = T // 512   # 4 query tiles of 512
JCn = T // 128   # 16 key chunks of 128
N_CORES = 8
RG = [[0, 1], [2, 3], [4, 5], [6, 7]]


def build_nc(with_rs: bool = True):
    nc = bacc.Bacc(None, target_bir_lowering=False)

    xT = nc.declare_dram_parameter("xT", [C, T], F32R, isOutput=False)
    wq = nc.declare_dram_parameter("wq", [C, 512], F32R, isOutput=False)
    wk = nc.declare_dram_parameter("wk", [C, 512], F32R, isOutput=False)
    wv = nc.declare_dram_parameter("wv", [C, 512], F32R, isOutput=False)
    wot = nc.declare_dram_parameter("wot", [512, C], F32R, isOutput=False)
    bo2 = nc.declare_dram_parameter("bo2", [128, 8], F32, isOutput=False)
    y = nc.declare_dram_parameter("y", [TTn, 512, 512], F32, isOutput=True)

    with TileContext(nc) as tc:
        with (
            tc.tile_pool(name="persist", bufs=1) as persist,
            tc.tile_pool(name="psum", bufs=1, space="PSUM") as psum,
            tc.tile_pool(name="dram", bufs=1, space="DRAM") as dram,
        ):
            # ---- persistent tiles ----
            qt = [persist.tile([128, T], F32R, tag=f"qt{p}", name=f"qt{p}")
                  for p in range(NPAIR)]
            kt = [persist.tile([128, T], F32R, tag=f"kt{p}", name=f"kt{p}")
                  for p in range(NPAIR)]
            # V chunks: 8 heads * 65 cols (64 d + ones col for free softmax sums)
            v = [persist.tile([128, 65 * HC], F32R, tag=f"v{j}", name=f"v{j}")
                 for j in range(JCn)]
            ones8 = persist.tile([128, HC], F32, tag="ones8")
            nc.vector.memset(ones8[:], 1.0)
            ones1f = persist.tile([1, 64], F32, tag="ones1f")
            nc.vector.memset(ones1f[:], 1.0)
            ones1 = persist.tile([1, 64], F32R, tag="ones1")
            nc.vector.tensor_copy(ones1[:], ones1f[:])
            bo_sb = persist.tile([128, 8], F32, tag="bo_sb")
            nc.sync.dma_start(out=bo_sb[:], in_=bo2[:])
            pt_pool = persist

            y_part = dram.tile([TTn, 1024, 512], F32)
            rs_out = dram.tile([TTn, 512, 512], F32)

            # ---- phase A: projections, streamed by t-slab ----
            with tc.tile_pool(name="xw", bufs=1) as xw:
                wqt = [xw.tile([128, 512], F32R, tag=f"wq{cc}", name=f"wq{cc}")
                       for cc in range(CCn)]
                wkt = [xw.tile([128, 512], F32R, tag=f"wk{cc}", name=f"wk{cc}")
                       for cc in range(CCn)]
                wvt = [xw.tile([128, 512], F32R, tag=f"wv{cc}", name=f"wv{cc}")
                       for cc in range(CCn)]
                def issue_xts(tt):
                    i0 = tt * 512
                    xts = [xw.tile([128, 512], F32R, tag=f"xt{cc}", bufs=2,
                                   name=f"xt{cc}_{tt}") for cc in range(CCn)]
                    for cc in range(CCn):
                        nc.sync.dma_start(
                            out=xts[cc][:], in_=xT[cc * 128:(cc + 1) * 128, i0:i0 + 512]
                        )
                    return xts

                for cc in range(CCn):
                    nc.sync.dma_start(out=wqt[cc][:], in_=wq[cc * 128:(cc + 1) * 128, :])
                xts0 = issue_xts(0)
                for cc in range(CCn):
                    nc.sync.dma_start(out=wkt[cc][:], in_=wk[cc * 128:(cc + 1) * 128, :])
                for cc in range(CCn):
                    nc.sync.dma_start(out=wvt[cc][:], in_=wv[cc * 128:(cc + 1) * 128, :])

                for tt in range(TTn):
                    i0 = tt * 512
                    xts = xts0 if tt == 0 else issue_xts(tt)
                    for wt, dst in ((wqt, qt), (wkt, kt)):
                        for p2 in range(NPAIR // 2):
                            ps = psum.tile([128, 1024], F32, tag="stps", bufs=2,
                                           name=f"aps{tt}{p2}")
                            for k in range(2):
                                p = 2 * p2 + k
                                for cc in range(CCn):
                                    nc.tensor.matmul(
                                        ps[:, k * 512:(k + 1) * 512],
                                        wt[cc][:, p * 128:(p + 1) * 128],
                                        xts[cc][:],
                                        start=(cc == 0), stop=(cc == CCn - 1),
                                        skip_group_check=True,
                                    )
                            for k in range(2):
                                nc.vector.tensor_copy(
                                    dst[2 * p2 + k][:, i0:i0 + 512],
                                    ps[:, k * 512:(k + 1) * 512],
                                )
                    for jc in range(4 * tt, 4 * tt + 4):
                        jl = jc * 128 - i0  # 0..383 within slab
                        ps = psum.tile([128, 512], F32, tag="ovps", bufs=2,
                                       name=f"vps{jc}")
                        for g in range(2):
                            for cc in range(CCn):
                                nc.tensor.matmul(
                                    ps[:, g * 256:(g + 1) * 256],
                                    xts[cc][:, jl:jl + 128],
                                    wvt[cc][:, g * 256:(g + 1) * 256],
                                    start=(cc == 0), stop=(cc == CCn - 1),
                                    skip_group_check=True,
                                )
                        dst_ap = v[jc][:].rearrange(
                            "p (h e) -> p h e", h=HC, e=65
                        )[:, :, 0:64]
                        nc.vector.tensor_copy(dst_ap, ps[:])
                        ones_ap = v[jc][:].rearrange(
                            "p (h e) -> p h e", h=HC, e=65
                        )[:, :, 64:65]
                        nc.vector.tensor_copy(ones_ap, ones8[:])

            # ---- phase B/C interleaved per tt ----
            with tc.tile_pool(name="bc_pool", bufs=1) as bcp:
                ot = [bcp.tile([128, T], F32R, tag=f"ot{p}", name=f"ot{p}")
                      for p in range(NPAIR)]
                wot_t = [bcp.tile([128, C], F32R, tag=f"wot{cl}", name=f"wot{cl}")
                         for cl in range(4)]
                for cl in range(4):
                    nc.sync.dma_start(
                        out=wot_t[cl][:], in_=wot[cl * 128:(cl + 1) * 128, :]
                    )

                held = None  # (ov, h, pt, kk, n_jc) AV group awaiting emission

                def emit_norm(pend):
                    nonlocal held
                    ov, p, e, i0 = pend
                    if held is not None and held[0] is ov:
                        emit_avs(held)
                        held = None
                    # rows 0:64 = unnormalized O^T, row 64 = softmax sum l
                    rl = bcp.tile([1, 512], F32, tag="rl", bufs=2)
                    nc.vector.reciprocal(rl[:], ov[64:65, :])
                    rlr = bcp.tile([1, 512], F32R, tag="rlr", bufs=2)
                    nc.vector.tensor_copy(rlr[:], rl[:])
                    bc = psum.tile([64, 512], F32, tag="yps", bufs=2)
                    nc.tensor.matmul(
                        bc[:], ones1[:], rlr[:], start=True, stop=True,
                        skip_group_check=True,
                    )
                    bc_sb = bcp.tile([64, 512], F32, tag="bc_sb", bufs=2)
                    nc.vector.tensor_copy(bc_sb[:], bc[:])
                    nc.vector.tensor_mul(
                        ot[p][e * 64:(e + 1) * 64, i0:i0 + 512],
                        ov[0:64, :], bc_sb[:],
                    )

                pending = None

                def emit_outproj_group(tt, cp):
                    i0 = tt * 512
                    yps = psum.tile([128, 512], F32, tag="yps", bufs=2,
                                    name=f"yps{tt}{cp}")
                    for cl in range(4):
                        nc.tensor.matmul(
                            yps[:],
                            wot_t[cl][:, cp * 128:(cp + 1) * 128],
                            ot[cl][:, i0:i0 + 512],
                            start=(cl == 0), stop=(cl == 3),
                            skip_group_check=True,
                        )
                    ysb = bcp.tile([128, 512], F32, tag="ysb", bufs=4)
                    nc.vector.tensor_scalar_add(ysb[:], yps[:], bo_sb[:, cp:cp + 1])
                    nc.sync.dma_start(
                        out=y_part[tt, cp * 128:(cp + 1) * 128, :], in_=ysb[:]
                    )

                def emit_rs(tt):
                    if with_rs:
                        nc.gpsimd.collective_compute(
                            "ReduceScatter",
                            mybir.AluOpType.add,
                            replica_groups=RG,
                            ins=[y_part[tt]],
                            outs=[rs_out[tt]],
                        )
                        nc.sync.dma_start(out=y[tt], in_=rs_out[tt])
                    else:
                        nc.sync.dma_start(out=y[tt], in_=y_part[tt, 0:512, :])

                def emit_avs(held):
                    ov, h, pt, kk, n_jc = held
                    for k in range(2):
                        jc, a = kk[k]
                        nc.tensor.matmul(
                            ov[:, a:512],
                            v[jc][:, h * 65:(h + 1) * 65],
                            pt[:, k * 512 + a:(k + 1) * 512],
                            start=(jc == 0), stop=(jc == n_jc - 1),
                            skip_group_check=True,
                        )

                for tt in range(TTn):
                    i0 = tt * 512
                    n_jc = 4 * (tt + 1)
                    for h in range(HC):
                        p, e = h // 2, h % 2
                        ov = psum.tile([65, 512], F32, tag="ovps", bufs=2,
                                      name=f"ov{tt}{h}")
                        for jc2 in range(n_jc // 2):
                            st = psum.tile([128, 1024], F32, tag="stps", bufs=2,
                                          name=f"st{tt}{h}{jc2}")
                            kk = []  # (jc, a) for the two chunks
                            for k in range(2):
                                jc = 2 * jc2 + k
                                kb = jc - 4 * tt  # band offset (>=0 within band)
                                a = min(kb * 128, 256) if kb >= 0 else 0
                                kk.append((jc, a))
                                nc.tensor.matmul(
                                    st[:, k * 512 + a:(k + 1) * 512],
                                    kt[p][e * 64:(e + 1) * 64,
                                          jc * 128:(jc + 1) * 128],
                                    qt[p][e * 64:(e + 1) * 64,
                                          i0 + a:i0 + 512],
                                    start=True, stop=True,
                                    skip_group_check=True,
                                )
                            # AV of the previously-held group (keeps PE fed
                            # while ACT works on this group's exp); crosses
                            # head boundaries so head h+1's QK never waits on
                            # head h's last exp chain.
                            if held is not None:
                                emit_avs(held)
                                held = None
                            if jc2 == 0 and tt >= 1 and 1 <= h <= 2:
                                # previous slab's outproj, two groups per head
                                # over the first heads so its RS fires early
                                # enough to overlap this slab's compute instead
                                # of stacking behind the next RS. norm(tt-1,h7)
                                # lands at (tt,h0,jc2==1), before these reads.
                                for g4 in range(4):
                                    emit_outproj_group(tt - 1, 4 * (h - 1) + g4)
                                if h == 2:
                                    emit_rs(tt - 1)
                            if jc2 == 1 and pending is not None:
                                emit_norm(pending)
                                pending = None
                            pt = pt_pool.tile([128, 1024], F32R, tag="pt", bufs=7,
                                              name=f"pt{tt}{h}{jc2}")
                            a0 = kk[0][1]
                            nc.scalar.activation(
                                pt[:, a0:1024], st[:, a0:1024],
                                mybir.ActivationFunctionType.Exp,
                            )
                            if kk[0][1] == 256:
                                # merged causal zeroing for band pair (k2,k3):
                                # slices [256:512],[768:1024]; iota = f - p - 128*o
                                sel = pt[:].rearrange(
                                    "p (o i) -> p o i", o=2, i=512)[:, :, 256:512]
                                nc.gpsimd.affine_select(
                                    out=sel, in_=sel,
                                    compare_op=mybir.AluOpType.is_ge,
                                    fill=0.0, base=0,
                                    pattern=[[-128, 2], [1, 256]],
                                    channel_multiplier=-1,
                                )
                            else:
                                for k in range(2):
                                    jc, a = kk[k]
                                    if jc >= 4 * tt:  # diag band chunk
                                        nc.gpsimd.affine_select(
                                            out=pt[:, k * 512 + a:(k + 1) * 512],
                                            in_=pt[:, k * 512 + a:(k + 1) * 512],
                                            compare_op=mybir.AluOpType.is_ge,
                                            fill=0.0,
                                            base=a - (jc - 4 * tt) * 128,
                                            pattern=[[1, 512 - a]],
                                            channel_multiplier=-1,
                                        )
                            held = (ov, h, pt, kk, n_jc)
                        if pending is not None:  # tt0 heads have only 2 groups
                            emit_norm(pending)
                        pending = (ov, p, e, i0)
                    if tt == TTn - 1:
                        if pending is not None:
                            emit_norm(pending)
                            pending = None
                        for cp in range(8):
                            emit_outproj_group(tt, cp)
                        emit_rs(tt)

    nc.compile()
    return nc


_NC_CACHE = {}


def _get_nc(with_rs: bool = True):
    key = bool(with_rs)
    if key not in _NC_CACHE:
        _NC_CACHE[key] = build_nc(with_rs)
    return _NC_CACHE[key]


def make_in_maps(x, Wq, Wk, Wv, Wo, bo):
    x = np.asarray(x, dtype=np.float32)
    Wq = np.asarray(Wq, dtype=np.float32)
    Wk = np.asarray(Wk, dtype=np.float32)
    Wv = np.asarray(Wv, dtype=np.float32)
    Wo = np.asarray(Wo, dtype=np.float32)
    bo = np.asarray(bo, dtype=np.float32)

    scale = np.float32(C) ** np.float32(-0.5)
    in_maps = []
    for c in range(N_CORES):
        b, hoff = c // 2, (c % 2) * HC
        heads = slice(hoff, hoff + HC)
        xT_c = np.ascontiguousarray(x[b].T)                      # [C, T]
        wq_c = np.ascontiguousarray(
            np.concatenate(list(Wq[heads] * scale), axis=1))     # [C, 512]
        wk_c = np.ascontiguousarray(np.concatenate(list(Wk[heads]), axis=1))
        wv_c = np.ascontiguousarray(np.concatenate(list(Wv[heads]), axis=1))
        wot_c = np.ascontiguousarray(Wo[:, hoff * D:(hoff + HC) * D].T)  # [512, C]
        bo2_c = np.ascontiguousarray((bo / 2.0).reshape(8, 128).T)       # [128, 8]
        in_maps.append({
            "xT": xT_c, "wq": wq_c, "wk": wk_c, "wv": wv_c,
            "wot": wot_c, "bo2": bo2_c,
        })
    return in_maps


def kernel(x, Wq, Wk, Wv, Wo, bo):
    nc = _get_nc(with_rs=True)
    in_maps = make_in_maps(x, Wq, Wk, Wv, Wo, bo)
    # The axon-tunneled devices occasionally fail transiently
    # (NRT_EXEC_UNIT_UNRECOVERABLE / tunnel hangup); a retry recovers.
    last_err = None
    for _ in range(3):
        try:
            res = run_bass_kernel_spmd(nc, in_maps, list(range(N_CORES))).results
            break
        except Exception as e:  # noqa: BLE001
            last_err = e
            import time
            time.sleep(5)
    else:
        raise last_err

    out = np.empty((B, T, C), dtype=np.float32)
    for c in range(N_CORES):
        b, e = c // 2, c % 2
        yc = res[c]["y"]  # [4, 512, 512] = [tt, c' slab, t]
        for tt in range(TTn):
            out[b, tt * 512:(tt + 1) * 512, e * 512:(e + 1) * 512] = yc[tt].T
    return out



# revision 4
# speedup vs baseline: 1.4040x; 1.4040x over previous
"""Multi-head causal attention (B=4, T=2048, C=1024, H=16, D=64) on 8 trn2 cores.

Sharding: tensor-parallel over heads within batch core-pairs.
  core c -> batch b = c//2, heads hoff..hoff+7 where hoff = (c%2)*8.

Per-core pipeline (all phases interleaved per 512-token slab tt):
  - Q/K projections in fp8e4 DoubleRow (2 k-tiles of 128 = 256-deep
    contraction per matmul), V projection in fp16.
  - QK^T per head in S^T = [key j, query i] orientation, fp8 DoubleRow with a
    stride-0 broadcast k-tile (doubles the product; folded into the exp scale
    together with the C**-0.5 softmax scale).
  - Causal masking via a [128,128] triangle(-30000) constant accumulated into
    the scores PSUM through an identity matmul before exp; exp then emits
    exact zeros for masked entries.  exp without max-subtraction (scores are
    ~N(0, 0.25^2), safe).
  - AV in fp16 with a ones column folded into V for free softmax sums;
    normalization = DVE reciprocal + Pool partition_broadcast + DVE multiply.
  - Output projection in fp16 to partial y^T [1024 c', 512 t] per slab
    (+ bo/2), pairwise fp16 ReduceScatter per slab; core even keeps
    c' 0:512, odd keeps 512:1024.
  - Projections for slab tt+1 and outproj/ReduceScatter for slab tt-1 are
    emitted inside slab tt's attention head loop to keep the PE dense.
Host reassembles the [B, T, C] output by transposing/concatenating slabs.
"""

import numpy as np

import concourse.bass as bass
import concourse.mybir as mybir
from concourse import bacc
from concourse.tile import TileContext
from concourse.bass_utils import run_bass_kernel_spmd

F32 = mybir.dt.float32
F16 = mybir.dt.float16
F8 = mybir.dt.float8e4
DRMODE = mybir.MatmulPerfMode.DoubleRow

B, T, C = 4, 2048, 1024
H, D = 16, 64
HC = 8            # heads per core
NPAIR = HC // 2   # head pairs (2x64 rows -> 128 partitions)
CC2 = 4           # 256-deep contraction pair-chunks for DoubleRow
CCn = 8           # 128-deep contraction chunks (fp16 path)
TTn = T // 512    # 4 query slabs of 512
JCn = T // 128    # 16 key chunks of 128
N_CORES = 8
RG = [[0, 1], [2, 3], [4, 5], [6, 7]]
MASKV = -30000.0
SC = 1.0 / 64.0   # C**-0.5 (=1/32) / 2 (stride-0 DoubleRow double-read)


def dr2(ap, n):
    """[P, W] -> [P, 2, W] with a stride-0 k-tile dim (double-read trick)."""
    return ap.unsqueeze(1).broadcast_to([ap.shape[0], n, ap.shape[1]])


def build_nc(with_rs: bool = True):
    nc = bacc.Bacc(None, target_bir_lowering=False)

    x8 = nc.declare_dram_parameter("x8", [CC2, 128, 2, T], F8, isOutput=False)
    x16 = nc.declare_dram_parameter("x16", [CCn, 128, T], F16, isOutput=False)
    wq8 = nc.declare_dram_parameter("wq8", [CC2, 128, 1024], F8, isOutput=False)
    wk8 = nc.declare_dram_parameter("wk8", [CC2, 128, 1024], F8, isOutput=False)
    wv16 = nc.declare_dram_parameter("wv16", [CCn, 128, 512], F16, isOutput=False)
    wot16 = nc.declare_dram_parameter("wot16", [4, 128, 1024], F16, isOutput=False)
    bo2 = nc.declare_dram_parameter("bo2", [128, 8], F32, isOutput=False)
    t128 = nc.declare_dram_parameter("t128", [128, 128], F16, isOutput=False)
    i128 = nc.declare_dram_parameter("i128", [128, 128], F16, isOutput=False)
    y = nc.declare_dram_parameter("y", [TTn, 512, 512], F16, isOutput=True)

    with TileContext(nc) as tc:
        with (
            tc.tile_pool(name="persist", bufs=1) as pp,
            tc.tile_pool(name="psum", bufs=1, space="PSUM") as psum,
            tc.tile_pool(name="dram", bufs=1, space="DRAM") as dram,
        ):
            # ---- persistent SBUF ----
            wq8_t = pp.tile([128, 4096], F8, tag="wq8")
            wk8_t = pp.tile([128, 4096], F8, tag="wk8")
            wv16_t = pp.tile([128, 4096], F16, tag="wv16")
            wot16_t = pp.tile([128, 4096], F16, tag="wot16")
            bo_sb = pp.tile([128, 8], F32, tag="bo_sb")
            t128_t = pp.tile([128, 128], F16, tag="t128")
            i128_t = pp.tile([128, 128], F16, tag="i128")
            qt8 = [pp.tile([128, T], F8, tag=f"qt{p}", name=f"qt{p}")
                   for p in range(NPAIR)]
            kt8 = [pp.tile([128, T], F8, tag=f"kt{p}", name=f"kt{p}")
                   for p in range(NPAIR)]
            # V: 8 heads * 65 cols (64 d + ones col for free softmax sums)
            v16 = [pp.tile([128, 65 * HC], F16, tag=f"v{j}", name=f"v{j}")
                   for j in range(JCn)]
            ot16 = [pp.tile([128, T], F16, tag=f"ot{p}", name=f"ot{p}")
                    for p in range(NPAIR)]

            y_part = dram.tile([TTn, 1024, 512], F16)
            rs_out = dram.tile([TTn, 512, 512], F16)

            # constants + weights first (tiny DMAs the first matmuls wait on)
            nc.sync.dma_start(out=t128_t[:], in_=t128[:, :])
            nc.sync.dma_start(out=i128_t[:], in_=i128[:, :])
            nc.sync.dma_start(
                out=wq8_t[:].rearrange("p (c f) -> p c f", c=CC2),
                in_=wq8[:, :, :].rearrange("c p f -> p c f"),
            )

            # ones columns of V (Pool memsets; d-cols overwritten by V proj)
            for jc in range(JCn):
                nc.gpsimd.memset(v16[jc][:], 1.0)

            # ---- A-phase emitters ----
            def dma_slab(tt):
                i0 = tt * 512
                xp = pp.tile([128, 4096], F8, tag="xp8", bufs=2,
                             name=f"xp8_{tt}")
                for cc2 in range(CC2):
                    nc.sync.dma_start(
                        out=xp[:, cc2 * 1024:(cc2 + 1) * 1024].rearrange(
                            "p (k t) -> p k t", k=2),
                        in_=x8[cc2, :, :, i0:i0 + 512],
                    )
                xs = pp.tile([128, 4096], F16, tag="x16s", bufs=2,
                             name=f"x16s_{tt}")
                nc.sync.dma_start(
                    out=xs[:].rearrange("p (c t) -> p c t", c=CCn),
                    in_=x16[:, :, i0:i0 + 512].rearrange("c p t -> p c t"),
                )
                return xp, xs

            def emit_projqk(tt, p, xp, wt, dst):
                i0 = tt * 512
                ps = psum.tile([128, 512], F32, tag="sm", bufs=2,
                               name=f"pqk{tt}{p}")
                for cc2 in range(CC2):
                    lhsT = wt[:, cc2 * 1024:(cc2 + 1) * 1024].rearrange(
                        "p (k m) -> p k m", k=2)[:, :, p * 128:(p + 1) * 128]
                    rhs = xp[:, cc2 * 1024:(cc2 + 1) * 1024].rearrange(
                        "p (k t) -> p k t", k=2)
                    nc.tensor.matmul(
                        ps[:], lhsT, rhs, start=(cc2 == 0), stop=(cc2 == CC2 - 1),
                        perf_mode=DRMODE, skip_group_check=True,
                    )
                nc.vector.tensor_copy(dst[p][:, i0:i0 + 512], ps[:])

            def emit_projv(tt, jc4, xs):
                jc = 4 * tt + jc4
                jl = jc4 * 128
                ps = psum.tile([128, 512], F32, tag="sm", bufs=2,
                               name=f"pv{jc}")
                for cc in range(CCn):
                    nc.tensor.matmul(
                        ps[:], xs[:, cc * 512 + jl:cc * 512 + jl + 128],
                        wv16_t[:, cc * 512:(cc + 1) * 512],
                        start=(cc == 0), stop=(cc == CCn - 1),
                        skip_group_check=True,
                    )
                dst = v16[jc][:].rearrange("p (h e) -> p h e", h=HC, e=65)
                nc.vector.tensor_copy(dst[:, :, 0:64], ps[:])

            # ---- B/C-phase emitters ----
            held = None     # (ov, h, pt, kk, n_jc) AV group awaiting emission
            pending = None  # (ov, p, e, i0) normalization awaiting emission

            def emit_avs(hd):
                ov, h, pt_, kk, n_jc = hd
                for k in range(2):
                    jc, a = kk[k]
                    nc.tensor.matmul(
                        ov[:, a:512], v16[jc][:, h * 65:(h + 1) * 65],
                        pt_[:, k * 512 + a:(k + 1) * 512],
                        start=(jc == 0), stop=(jc == n_jc - 1),
                        skip_group_check=True,
                    )

            def emit_norm(pend):
                nonlocal held
                ov, p, e, i0 = pend
                if held is not None and held[0] is ov:
                    emit_avs(held)
                    held = None
                rl = pp.tile([1, 512], F32, tag="rl", bufs=4)
                nc.vector.reciprocal(rl[:], ov[64:65, :])
                bcb = pp.tile([64, 512], F32, tag="bcb", bufs=4)
                nc.gpsimd.partition_broadcast(bcb[:], rl[:])
                nc.vector.tensor_mul(
                    ot16[p][e * 64:(e + 1) * 64, i0:i0 + 512],
                    ov[0:64, :], bcb[:],
                )

            def emit_outproj(tt, cp):
                i0 = tt * 512
                yps = psum.tile([128, 512], F32, tag="sm", bufs=2,
                                name=f"yps{tt}{cp}")
                for cl in range(4):
                    nc.tensor.matmul(
                        yps[:],
                        wot16_t[:, cl * 1024 + cp * 128:cl * 1024 + (cp + 1) * 128],
                        ot16[cl][:, i0:i0 + 512],
                        start=(cl == 0), stop=(cl == 3),
                        skip_group_check=True,
                    )
                ysb = pp.tile([128, 512], F16, tag="ysb", bufs=4)
                nc.vector.tensor_scalar_add(ysb[:], yps[:], bo_sb[:, cp:cp + 1])
                nc.sync.dma_start(
                    out=y_part[tt, cp * 128:(cp + 1) * 128, :], in_=ysb[:]
                )

            def emit_rs(tt):
                if with_rs:
                    nc.gpsimd.collective_compute(
                        "ReduceScatter", mybir.AluOpType.add,
                        replica_groups=RG,
                        ins=[y_part[tt]], outs=[rs_out[tt]],
                    )
                    nc.sync.dma_start(out=y[tt], in_=rs_out[tt])
                else:
                    nc.sync.dma_start(out=y[tt], in_=y_part[tt, 0:512, :])

            # ---- filler scheduling ----
            slabs = {}

            def run_filler(f):
                kind = f[0]
                if kind == "op":
                    emit_outproj(f[1], f[2])
                elif kind == "rs":
                    emit_rs(f[1])
                elif kind == "dma":
                    slabs[f[1]] = dma_slab(f[1])
                elif kind == "pq":
                    emit_projqk(f[1], f[2], slabs[f[1]][0], wq8_t, qt8)
                elif kind == "pk":
                    emit_projqk(f[1], f[2], slabs[f[1]][0], wk8_t, kt8)
                elif kind == "pv":
                    emit_projv(f[1], f[2], slabs[f[1]][1])

            def build_fillers(tt):
                fl = []
                if tt >= 1:
                    fl += [("op", tt - 1, cp) for cp in range(8)]
                    fl.append(("rs", tt - 1))
                if tt + 1 < TTn:
                    fl.append(("dma", tt + 1))
                    fl += [("pq", tt + 1, p) for p in range(NPAIR)]
                    fl += [("pk", tt + 1, p) for p in range(NPAIR)]
                    fl += [("pv", tt + 1, j) for j in range(4)]
                return fl

            # ---- remaining initial loads + phase A for slab 0 ----
            slabs[0] = dma_slab(0)
            nc.sync.dma_start(
                out=wk8_t[:].rearrange("p (c f) -> p c f", c=CC2),
                in_=wk8[:, :, :].rearrange("c p f -> p c f"),
            )
            nc.sync.dma_start(
                out=wv16_t[:].rearrange("p (c f) -> p c f", c=CCn),
                in_=wv16[:, :, :].rearrange("c p f -> p c f"),
            )
            nc.sync.dma_start(
                out=wot16_t[:].rearrange("p (c f) -> p c f", c=4),
                in_=wot16[:, :, :].rearrange("c p f -> p c f"),
            )
            nc.sync.dma_start(out=bo_sb[:], in_=bo2[:, :])
            for p in range(NPAIR):
                emit_projqk(0, p, slabs[0][0], wq8_t, qt8)
                emit_projqk(0, p, slabs[0][0], wk8_t, kt8)
            for j in range(4):
                emit_projv(0, j, slabs[0][1])

            # ---- main loop over slabs ----
            for tt in range(TTn):
                i0 = tt * 512
                n_jc = 4 * (tt + 1)
                fillers = build_fillers(tt)
                fi = 0
                for h in range(HC):
                    p, e = h // 2, h % 2
                    ov = psum.tile([65, 512], F32, tag="ov", bufs=2,
                                   name=f"ov{tt}{h}")
                    for jc2 in range(n_jc // 2):
                        st = psum.tile([128, 1024], F32, tag="st", bufs=2,
                                       name=f"st{tt}{h}{jc2}")
                        kk = []
                        for k in range(2):
                            jc = 2 * jc2 + k
                            kb = jc - 4 * tt
                            a = kb * 128 if kb >= 0 else 0
                            kk.append((jc, a))
                            lhsT = dr2(
                                kt8[p][e * 64:(e + 1) * 64,
                                       jc * 128:(jc + 1) * 128], 2)
                            rhs = dr2(
                                qt8[p][e * 64:(e + 1) * 64, i0 + a:i0 + 512], 2)
                            nc.tensor.matmul(
                                st[:, k * 512 + a:(k + 1) * 512], lhsT, rhs,
                                start=True, stop=(kb < 0),
                                perf_mode=DRMODE, skip_group_check=True,
                            )
                            if kb >= 0:
                                # causal triangle mask add on the diag block
                                nc.tensor.matmul(
                                    st[:, k * 512 + a:k * 512 + a + 128],
                                    i128_t[:], t128_t[:],
                                    start=False, stop=True,
                                    skip_group_check=True,
                                )
                        # AV of the previously-held group keeps the PE fed
                        # while ACT works on this group's exp; crosses head
                        # boundaries so head h+1's QK never waits on head h's
                        # last exp chain.
                        if held is not None:
                            emit_avs(held)
                            held = None
                        if jc2 == 1 and pending is not None:
                            emit_norm(pending)
                            pending = None
                        if jc2 >= 1 and (tt == 0 or h >= 1):
                            for _ in range(2):
                                if fi < len(fillers):
                                    run_filler(fillers[fi])
                                    fi += 1
                        pt_ = pp.tile([128, 1024], F16, tag="pt", bufs=7,
                                      name=f"pt{tt}{h}{jc2}")
                        a0 = kk[0][1]
                        nc.scalar.activation(
                            pt_[:, a0:1024], st[:, a0:1024],
                            mybir.ActivationFunctionType.Exp, scale=SC,
                        )
                        held = (ov, h, pt_, kk, n_jc)
                    if pending is not None:
                        emit_norm(pending)
                        pending = None
                    pending = (ov, p, e, i0)
                while fi < len(fillers):
                    run_filler(fillers[fi])
                    fi += 1
                if tt == TTn - 1:
                    if pending is not None:
                        emit_norm(pending)
                        pending = None
                    for cp in range(8):
                        emit_outproj(tt, cp)
                    emit_rs(tt)

    nc.compile()
    return nc


_NC_CACHE = {}


def _get_nc(with_rs: bool = True):
    key = bool(with_rs)
    if key not in _NC_CACHE:
        _NC_CACHE[key] = build_nc(with_rs)
    return _NC_CACHE[key]


def make_in_maps(x, Wq, Wk, Wv, Wo, bo):
    import ml_dtypes
    F8NP = ml_dtypes.float8_e4m3fn

    x = np.asarray(x, dtype=np.float32)
    Wq = np.asarray(Wq, dtype=np.float32)
    Wk = np.asarray(Wk, dtype=np.float32)
    Wv = np.asarray(Wv, dtype=np.float32)
    Wo = np.asarray(Wo, dtype=np.float32)
    bo = np.asarray(bo, dtype=np.float32)

    jj = np.arange(128)
    tri = np.where(jj[None, :] < jj[:, None], np.float32(MASKV), 0.0)
    tri = tri.astype(np.float16)          # t128[j, m] = MASKV if m < j
    eye = np.eye(128, dtype=np.float16)

    def dr_pack(w):  # [C, 512] -> [CC2, 128, 1024] with (k, m) free layout
        return np.ascontiguousarray(
            w.reshape(CC2, 2, 128, 512).transpose(0, 2, 1, 3).reshape(
                CC2, 128, 1024))

    in_maps = []
    for c in range(N_CORES):
        b, hoff = c // 2, (c % 2) * HC
        heads = slice(hoff, hoff + HC)
        xT = np.ascontiguousarray(x[b].T)                       # [C, T]
        x8_c = np.ascontiguousarray(
            xT.reshape(CC2, 2, 128, T).transpose(0, 2, 1, 3)).astype(F8NP)
        x16_c = np.ascontiguousarray(xT.reshape(CCn, 128, T)).astype(np.float16)
        wq_c = dr_pack(np.concatenate(list(Wq[heads]), axis=1)).astype(F8NP)
        wk_c = dr_pack(np.concatenate(list(Wk[heads]), axis=1)).astype(F8NP)
        wv_c = np.ascontiguousarray(
            np.concatenate(list(Wv[heads]), axis=1).reshape(
                CCn, 128, 512)).astype(np.float16)
        wot_c = np.ascontiguousarray(
            Wo[:, hoff * D:(hoff + HC) * D].T.reshape(
                4, 128, 1024)).astype(np.float16)
        bo2_c = np.ascontiguousarray((bo / 2.0).reshape(8, 128).T)  # [128, 8]
        in_maps.append({
            "x8": x8_c, "x16": x16_c, "wq8": wq_c, "wk8": wk_c,
            "wv16": wv_c, "wot16": wot_c, "bo2": bo2_c,
            "t128": tri, "i128": eye,
        })
    return in_maps


def kernel(x, Wq, Wk, Wv, Wo, bo):
    nc = _get_nc(with_rs=True)
    in_maps = make_in_maps(x, Wq, Wk, Wv, Wo, bo)
    # The axon-tunneled devices occasionally fail transiently
    # (NRT_EXEC_UNIT_UNRECOVERABLE / tunnel hangup); a retry recovers.
    last_err = None
    for _ in range(3):
        try:
            res = run_bass_kernel_spmd(nc, in_maps, list(range(N_CORES))).results
            break
        except Exception as e:  # noqa: BLE001
            last_err = e
            import time
            time.sleep(5)
    else:
        raise last_err

    out = np.empty((B, T, C), dtype=np.float32)
    for c in range(N_CORES):
        b, e = c // 2, c % 2
        yc = np.asarray(res[c]["y"]).astype(np.float32)  # [tt, c' slab, t]
        for tt in range(TTn):
            out[b, tt * 512:(tt + 1) * 512, e * 512:(e + 1) * 512] = yc[tt].T
    return out


# revision 6
# speedup vs baseline: 1.4686x; 1.0460x over previous
"""Multi-head causal attention (B=4, T=2048, C=1024, H=16, D=64) on 8 trn2 cores.

Sharding: tensor-parallel over heads within batch core-pairs.
  core c -> batch b = c//2, heads hoff..hoff+7 where hoff = (c%2)*8.

Per-core pipeline (all phases interleaved per 512-token slab tt):
  - Q/K projections in fp8e4 DoubleRow (2 k-tiles of 128 = 256-deep
    contraction per matmul), V projection in fp16.
  - QK^T per head in S^T = [key j, query i] orientation, fp8 DoubleRow with a
    stride-0 broadcast k-tile (doubles the product; folded into the exp scale
    together with the C**-0.5 softmax scale).
  - Causal masking via a [128,128] triangle(-30000) constant accumulated into
    the scores PSUM through an identity matmul before exp; exp then emits
    exact zeros for masked entries.  exp without max-subtraction (scores are
    ~N(0, 0.25^2), safe).
  - AV in fp16 with a ones column folded into V for free softmax sums;
    normalization = DVE reciprocal + Pool partition_broadcast + DVE multiply.
  - Output projection in fp16 to partial y^T [1024 c', 512 t] per slab
    (+ bo/2), pairwise fp16 ReduceScatter per slab; core even keeps
    c' 0:512, odd keeps 512:1024.
  - Projections for slab tt+1 and outproj/ReduceScatter for slab tt-1 are
    emitted inside slab tt's attention head loop to keep the PE dense.
Host reassembles the [B, T, C] output by transposing/concatenating slabs.
"""

import numpy as np

import concourse.bass as bass
import concourse.mybir as mybir
from concourse import bacc
from concourse.tile import TileContext
from concourse.bass_utils import run_bass_kernel_spmd

F32 = mybir.dt.float32
F16 = mybir.dt.float16
F8 = mybir.dt.float8e4
DRMODE = mybir.MatmulPerfMode.DoubleRow

B, T, C = 4, 2048, 1024
H, D = 16, 64
HC = 8            # heads per core
NPAIR = HC // 2   # head pairs (2x64 rows -> 128 partitions)
CC2 = 4           # 256-deep contraction pair-chunks for DoubleRow
CCn = 8           # 128-deep contraction chunks (fp16 path)
TTn = T // 512    # 4 query slabs of 512
JCn = T // 128    # 16 key chunks of 128
N_CORES = 8
RG = [[0, 1], [2, 3], [4, 5], [6, 7]]
MASKV = -30000.0
SC = 1.0 / 64.0   # C**-0.5 (=1/32) / 2 (stride-0 DoubleRow double-read)


def dr2(ap, n):
    """[P, W] -> [P, 2, W] with a stride-0 k-tile dim (double-read trick)."""
    return ap.unsqueeze(1).broadcast_to([ap.shape[0], n, ap.shape[1]])


def build_nc(with_rs: bool = True):
    nc = bacc.Bacc(None, target_bir_lowering=False)

    x8 = nc.declare_dram_parameter("x8", [CC2, 128, 2, T], F8, isOutput=False)
    x16 = nc.declare_dram_parameter("x16", [CCn, 128, T], F16, isOutput=False)
    wq8 = nc.declare_dram_parameter("wq8", [CC2, 128, 1024], F8, isOutput=False)
    wk8 = nc.declare_dram_parameter("wk8", [CC2, 128, 1024], F8, isOutput=False)
    wv16 = nc.declare_dram_parameter("wv16", [CCn, 128, 512], F16, isOutput=False)
    wot16 = nc.declare_dram_parameter("wot16", [4, 128, 1024], F16, isOutput=False)
    bo2 = nc.declare_dram_parameter("bo2", [128, 8], F32, isOutput=False)
    t128 = nc.declare_dram_parameter("t128", [128, 128], F16, isOutput=False)
    i128 = nc.declare_dram_parameter("i128", [128, 128], F16, isOutput=False)
    y = nc.declare_dram_parameter("y", [TTn, 512, 512], F16, isOutput=True)

    with TileContext(nc) as tc:
        with (
            tc.tile_pool(name="persist", bufs=1) as pp,
            tc.tile_pool(name="psum", bufs=1, space="PSUM") as psum,
            tc.tile_pool(name="dram", bufs=1, space="DRAM") as dram,
        ):
            # ---- persistent SBUF ----
            wq8_t = pp.tile([128, 4096], F8, tag="wq8")
            wk8_t = pp.tile([128, 4096], F8, tag="wk8")
            wv16_t = pp.tile([128, 4096], F16, tag="wv16")
            wot16_t = pp.tile([128, 4096], F16, tag="wot16")
            bo_sb = pp.tile([128, 8], F32, tag="bo_sb")
            t128_t = pp.tile([128, 128], F16, tag="t128")
            i128_t = pp.tile([128, 128], F16, tag="i128")
            qt8 = [pp.tile([128, T], F8, tag=f"qt{p}", name=f"qt{p}")
                   for p in range(NPAIR)]
            kt8 = [pp.tile([128, T], F8, tag=f"kt{p}", name=f"kt{p}")
                   for p in range(NPAIR)]
            # V: 8 heads * 65 cols (64 d + ones col for free softmax sums)
            v16 = [pp.tile([128, 65 * HC], F16, tag=f"v{j}", name=f"v{j}")
                   for j in range(JCn)]
            ot16 = [pp.tile([128, T], F16, tag=f"ot{p}", name=f"ot{p}")
                    for p in range(NPAIR)]

            y_part = dram.tile([TTn, 1024, 512], F16)
            rs_out = dram.tile([TTn, 512, 512], F16)

            # constants + weights first (tiny DMAs the first matmuls wait on)
            nc.sync.dma_start(out=t128_t[:], in_=t128[:, :])
            nc.sync.dma_start(out=i128_t[:], in_=i128[:, :])
            nc.sync.dma_start(
                out=wq8_t[:].rearrange("p (c f) -> p c f", c=CC2),
                in_=wq8[:, :, :].rearrange("c p f -> p c f"),
            )

            # ones columns of V (Pool memsets; d-cols overwritten by V proj)
            for jc in range(JCn):
                nc.gpsimd.memset(v16[jc][:], 1.0)

            # ---- A-phase emitters ----
            def dma_slab(tt):
                i0 = tt * 512
                xp = pp.tile([128, 4096], F8, tag="xp8", bufs=2,
                             name=f"xp8_{tt}")
                for cc2 in range(CC2):
                    nc.sync.dma_start(
                        out=xp[:, cc2 * 1024:(cc2 + 1) * 1024].rearrange(
                            "p (k t) -> p k t", k=2),
                        in_=x8[cc2, :, :, i0:i0 + 512],
                    )
                xs = pp.tile([128, 4096], F16, tag="x16s", bufs=2,
                             name=f"x16s_{tt}")
                nc.sync.dma_start(
                    out=xs[:].rearrange("p (c t) -> p c t", c=CCn),
                    in_=x16[:, :, i0:i0 + 512].rearrange("c p t -> p c t"),
                )
                return xp, xs

            def emit_projqk(tt, p, xp, wt, dst):
                i0 = tt * 512
                ps = psum.tile([128, 512], F32, tag="sm", bufs=2,
                               name=f"pqk{tt}{p}")
                for cc2 in range(CC2):
                    lhsT = wt[:, cc2 * 1024:(cc2 + 1) * 1024].rearrange(
                        "p (k m) -> p k m", k=2)[:, :, p * 128:(p + 1) * 128]
                    rhs = xp[:, cc2 * 1024:(cc2 + 1) * 1024].rearrange(
                        "p (k t) -> p k t", k=2)
                    nc.tensor.matmul(
                        ps[:], lhsT, rhs, start=(cc2 == 0), stop=(cc2 == CC2 - 1),
                        perf_mode=DRMODE, skip_group_check=True,
                    )
                nc.vector.tensor_copy(dst[p][:, i0:i0 + 512], ps[:])

            def emit_projv(tt, jc4, xs):
                jc = 4 * tt + jc4
                jl = jc4 * 128
                ps = psum.tile([128, 512], F32, tag="sm", bufs=2,
                               name=f"pv{jc}")
                for cc in range(CCn):
                    nc.tensor.matmul(
                        ps[:], xs[:, cc * 512 + jl:cc * 512 + jl + 128],
                        wv16_t[:, cc * 512:(cc + 1) * 512],
                        start=(cc == 0), stop=(cc == CCn - 1),
                        skip_group_check=True,
                    )
                dst = v16[jc][:].rearrange("p (h e) -> p h e", h=HC, e=65)
                nc.vector.tensor_copy(dst[:, :, 0:64], ps[:])

            # ---- B/C-phase emitters ----
            # AV groups awaiting emission, depth 2: AV(g) is emitted during
            # group g+2's QK so exp(g) has two full group-times to complete
            # before the PE reaches AV(g).
            held = []
            pending = None  # (ov, p, e, i0) normalization awaiting emission

            def emit_avs(hd):
                ov, h, pt_, kk, n_jc = hd
                for k in range(2):
                    jc, a = kk[k]
                    nc.tensor.matmul(
                        ov[:, a:512], v16[jc][:, h * 65:(h + 1) * 65],
                        pt_[:, k * 512 + a:(k + 1) * 512],
                        start=(jc == 0), stop=(jc == n_jc - 1),
                        skip_group_check=True,
                    )

            def emit_norm(pend):
                ov, p, e, i0 = pend
                while held and any(hd[0] is ov for hd in held):
                    emit_avs(held.pop(0))
                rl = pp.tile([1, 512], F32, tag="rl", bufs=4)
                nc.vector.reciprocal(rl[:], ov[64:65, :])
                bcb = pp.tile([64, 512], F32, tag="bcb", bufs=4)
                nc.gpsimd.partition_broadcast(bcb[:], rl[:])
                nc.vector.tensor_mul(
                    ot16[p][e * 64:(e + 1) * 64, i0:i0 + 512],
                    ov[0:64, :], bcb[:],
                )

            def emit_outproj(tt, cp):
                i0 = tt * 512
                yps = psum.tile([128, 512], F32, tag="sm", bufs=2,
                                name=f"yps{tt}{cp}")
                for cl in range(4):
                    nc.tensor.matmul(
                        yps[:],
                        wot16_t[:, cl * 1024 + cp * 128:cl * 1024 + (cp + 1) * 128],
                        ot16[cl][:, i0:i0 + 512],
                        start=(cl == 0), stop=(cl == 3),
                        skip_group_check=True,
                    )
                ysb = pp.tile([128, 512], F16, tag="ysb", bufs=4)
                nc.vector.tensor_scalar_add(ysb[:], yps[:], bo_sb[:, cp:cp + 1])
                nc.sync.dma_start(
                    out=y_part[tt, cp * 128:(cp + 1) * 128, :], in_=ysb[:]
                )

            def emit_rs(tt):
                if with_rs:
                    nc.gpsimd.collective_compute(
                        "ReduceScatter", mybir.AluOpType.add,
                        replica_groups=RG,
                        ins=[y_part[tt]], outs=[rs_out[tt]],
                    )
                    nc.sync.dma_start(out=y[tt], in_=rs_out[tt])
                else:
                    nc.sync.dma_start(out=y[tt], in_=y_part[tt, 0:512, :])

            # ---- filler scheduling ----
            slabs = {}

            def run_filler(f):
                kind = f[0]
                if kind == "op":
                    emit_outproj(f[1], f[2])
                elif kind == "rs":
                    emit_rs(f[1])
                elif kind == "dma":
                    slabs[f[1]] = dma_slab(f[1])
                elif kind == "pq":
                    emit_projqk(f[1], f[2], slabs[f[1]][0], wq8_t, qt8)
                elif kind == "pk":
                    emit_projqk(f[1], f[2], slabs[f[1]][0], wk8_t, kt8)
                elif kind == "pv":
                    emit_projv(f[1], f[2], slabs[f[1]][1])

            def build_fillers(tt):
                fl = []
                if tt >= 1:
                    fl += [("op", tt - 1, cp) for cp in range(8)]
                    fl.append(("rs", tt - 1))
                if tt + 1 < TTn:
                    fl.append(("dma", tt + 1))
                    fl += [("pq", tt + 1, p) for p in range(NPAIR)]
                    fl += [("pk", tt + 1, p) for p in range(NPAIR)]
                    fl += [("pv", tt + 1, j) for j in range(4)]
                return fl

            # ---- remaining initial loads + phase A for slab 0 ----
            slabs[0] = dma_slab(0)
            nc.sync.dma_start(
                out=wk8_t[:].rearrange("p (c f) -> p c f", c=CC2),
                in_=wk8[:, :, :].rearrange("c p f -> p c f"),
            )
            nc.sync.dma_start(
                out=wv16_t[:].rearrange("p (c f) -> p c f", c=CCn),
                in_=wv16[:, :, :].rearrange("c p f -> p c f"),
            )
            nc.sync.dma_start(
                out=wot16_t[:].rearrange("p (c f) -> p c f", c=4),
                in_=wot16[:, :, :].rearrange("c p f -> p c f"),
            )
            nc.sync.dma_start(out=bo_sb[:], in_=bo2[:, :])
            for p in range(NPAIR):
                emit_projqk(0, p, slabs[0][0], wq8_t, qt8)
                emit_projqk(0, p, slabs[0][0], wk8_t, kt8)
            for j in range(4):
                emit_projv(0, j, slabs[0][1])

            # ---- main loop over slabs ----
            for tt in range(TTn):
                i0 = tt * 512
                n_jc = 4 * (tt + 1)
                fillers = build_fillers(tt)
                fi = 0
                for h in range(HC):
                    p, e = h // 2, h % 2
                    ov = psum.tile([65, 512], F32, tag="ov", bufs=2,
                                   name=f"ov{tt}{h}")
                    for jc2 in range(n_jc // 2):
                        st = psum.tile([128, 1024], F32, tag="st", bufs=2,
                                       name=f"st{tt}{h}{jc2}")
                        kk = []
                        for k in range(2):
                            jc = 2 * jc2 + k
                            kb = jc - 4 * tt
                            a = kb * 128 if kb >= 0 else 0
                            kk.append((jc, a))
                            lhsT = dr2(
                                kt8[p][e * 64:(e + 1) * 64,
                                       jc * 128:(jc + 1) * 128], 2)
                            rhs = dr2(
                                qt8[p][e * 64:(e + 1) * 64, i0 + a:i0 + 512], 2)
                            nc.tensor.matmul(
                                st[:, k * 512 + a:(k + 1) * 512], lhsT, rhs,
                                start=True, stop=(kb < 0),
                                perf_mode=DRMODE, skip_group_check=True,
                            )
                            if kb >= 0:
                                # causal triangle mask add on the diag block
                                nc.tensor.matmul(
                                    st[:, k * 512 + a:k * 512 + a + 128],
                                    i128_t[:], t128_t[:],
                                    start=False, stop=True,
                                    skip_group_check=True,
                                )
                        # AV of group g-2 keeps the PE fed while ACT works on
                        # groups g-1/g's exp; crosses head boundaries so head
                        # h+1's QK never waits on head h's last exp chain.
                        if len(held) >= 2:
                            emit_avs(held.pop(0))
                        if jc2 == 1 and pending is not None:
                            emit_norm(pending)
                            pending = None
                        if jc2 >= (1 if (tt == 0 or h >= 1) else 2):
                            for _ in range(2 if tt == 0 else 1):
                                if fi < len(fillers):
                                    run_filler(fillers[fi])
                                    fi += 1
                        pt_ = pp.tile([128, 1024], F16, tag="pt", bufs=7,
                                      name=f"pt{tt}{h}{jc2}")
                        a0 = kk[0][1]
                        nc.scalar.activation(
                            pt_[:, a0:1024], st[:, a0:1024],
                            mybir.ActivationFunctionType.Exp, scale=SC,
                        )
                        held.append((ov, h, pt_, kk, n_jc))
                    if pending is not None:
                        emit_norm(pending)
                        pending = None
                    pending = (ov, p, e, i0)
                while fi < len(fillers):
                    run_filler(fillers[fi])
                    fi += 1
                if tt == TTn - 1:
                    if pending is not None:
                        emit_norm(pending)
                        pending = None
                    for cp in range(8):
                        emit_outproj(tt, cp)
                    emit_rs(tt)

    nc.compile()
    return nc


_NC_CACHE = {}


def _get_nc(with_rs: bool = True):
    key = bool(with_rs)
    if key not in _NC_CACHE:
        _NC_CACHE[key] = build_nc(with_rs)
    return _NC_CACHE[key]


def make_in_maps(x, Wq, Wk, Wv, Wo, bo):
    import ml_dtypes
    F8NP = ml_dtypes.float8_e4m3fn

    x = np.asarray(x, dtype=np.float32)
    Wq = np.asarray(Wq, dtype=np.float32)
    Wk = np.asarray(Wk, dtype=np.float32)
    Wv = np.asarray(Wv, dtype=np.float32)
    Wo = np.asarray(Wo, dtype=np.float32)
    bo = np.asarray(bo, dtype=np.float32)

    jj = np.arange(128)
    tri = np.where(jj[None, :] < jj[:, None], np.float32(MASKV), 0.0)
    tri = tri.astype(np.float16)          # t128[j, m] = MASKV if m < j
    eye = np.eye(128, dtype=np.float16)

    def dr_pack(w):  # [C, 512] -> [CC2, 128, 1024] with (k, m) free layout
        return np.ascontiguousarray(
            w.reshape(CC2, 2, 128, 512).transpose(0, 2, 1, 3).reshape(
                CC2, 128, 1024))

    in_maps = []
    for c in range(N_CORES):
        b, hoff = c // 2, (c % 2) * HC
        heads = slice(hoff, hoff + HC)
        xT = np.ascontiguousarray(x[b].T)                       # [C, T]
        x8_c = np.ascontiguousarray(
            xT.reshape(CC2, 2, 128, T).transpose(0, 2, 1, 3)).astype(F8NP)
        x16_c = np.ascontiguousarray(xT.reshape(CCn, 128, T)).astype(np.float16)
        wq_c = dr_pack(np.concatenate(list(Wq[heads]), axis=1)).astype(F8NP)
        wk_c = dr_pack(np.concatenate(list(Wk[heads]), axis=1)).astype(F8NP)
        wv_c = np.ascontiguousarray(
            np.concatenate(list(Wv[heads]), axis=1).reshape(
                CCn, 128, 512)).astype(np.float16)
        wot_c = np.ascontiguousarray(
            Wo[:, hoff * D:(hoff + HC) * D].T.reshape(
                4, 128, 1024)).astype(np.float16)
        bo2_c = np.ascontiguousarray((bo / 2.0).reshape(8, 128).T)  # [128, 8]
        in_maps.append({
            "x8": x8_c, "x16": x16_c, "wq8": wq_c, "wk8": wk_c,
            "wv16": wv_c, "wot16": wot_c, "bo2": bo2_c,
            "t128": tri, "i128": eye,
        })
    return in_maps


def kernel(x, Wq, Wk, Wv, Wo, bo):
    nc = _get_nc(with_rs=True)
    in_maps = make_in_maps(x, Wq, Wk, Wv, Wo, bo)
    # The axon-tunneled devices occasionally fail transiently
    # (NRT_EXEC_UNIT_UNRECOVERABLE / tunnel hangup); a retry recovers.
    last_err = None
    for _ in range(3):
        try:
            res = run_bass_kernel_spmd(nc, in_maps, list(range(N_CORES))).results
            break
        except Exception as e:  # noqa: BLE001
            last_err = e
            import time
            time.sleep(5)
    else:
        raise last_err

    out = np.empty((B, T, C), dtype=np.float32)
    for c in range(N_CORES):
        b, e = c // 2, c % 2
        yc = np.asarray(res[c]["y"]).astype(np.float32)  # [tt, c' slab, t]
        for tt in range(TTn):
            out[b, tt * 512:(tt + 1) * 512, e * 512:(e + 1) * 512] = yc[tt].T
    return out
